# revision 9
# baseline (speedup 1.0000x reference)
"""Trainium2 Bass kernel for Mixtral-style attention (QKV proj + per-head
RMSNorm on Q/K + NeoX RoPE + GQA causal attention + output projection).

Sharding: tensor-parallel across heads — each of the 8 cores owns one GQA
group (4 Q heads + 1 KV head): Wqkv column-sharded, Wo row-sharded.  The
o_proj partial sums are reduced on the host (the natural "unshard" for
row-parallel Wo).

Device-side layout strategy per core:
  - host supplies hidden^T [H, T] so the QKV matmul contracts on partitions
    with zero on-device transposes of the activations
  - qkv computed in natural [t, c] layout -> RMSNorm reduction + RoPE pair
    shuffle live on the free axis (cheap DVE ops)
  - q/k transposed per 128-tile via the PE (identity matmul) into [d, t]
    layout; attention computes S^T = k_eff^T-contract-q_eff directly, so
    softmax normalization needs no P transposes:
      P = exp(S^T) (causal-masked), den = ones^T @ P (PE),
      oT = v^T-contract-P (PE), o = oT * (1/den broadcast via K=1 matmul)
  - no max-subtraction in softmax: post-RMSNorm scores are bounded by
    sqrt(d)*max|w|^2*scale ~= 12, far from fp32 overflow, and softmax is
    shift-invariant so the result is identical
  - matmuls run as float32r (TF32-like, 1 col/cycle at N>=256; 4x faster
    than fp32) with fp32 PSUM accumulation
"""

import numpy as np
from contextlib import ExitStack

import concourse.bass as bass
import concourse.tile as tile
from concourse import bacc, mybir
from concourse.bass_utils import run_bass_kernel_spmd

F32 = mybir.dt.float32
F32R = mybir.dt.float32r
AF = mybir.ActivationFunctionType

HIDDEN = 4096
HEAD_DIM = 128
N_HEADS = 32
N_KV_HEADS = 8
N_CORES = 8
QH = N_HEADS // N_CORES          # 4 q heads per core
ROPE_THETA = 1e6
EPS = 1e-6
HALF = HEAD_DIM // 2
NH = HIDDEN // 128               # 32 contraction tiles

# set by test harness to collect profile info
TRACE = False
LAST_RESULTS = None


def build_nc(T):
    NT = T // 128                 # t tiles
    NCH = T // 512                # 512-wide tq chunks
    assert T % 512 == 0

    nc = bacc.Bacc("TRN2", target_bir_lowering=False, debug=False,
                   num_devices=N_CORES)

    hT_d = nc.dram_tensor("hT", [HIDDEN, T], F32R, kind="ExternalInput").ap()
    wq_d = nc.dram_tensor("wqkv", [HIDDEN, 768], F32R, kind="ExternalInput").ap()
    wo_d = nc.dram_tensor("wo", [512, HIDDEN], F32R, kind="ExternalInput").ap()
    cwq_d = nc.dram_tensor("cwq", [T, 128], F32, kind="ExternalInput").ap()
    swq_d = nc.dram_tensor("swq", [T, 128], F32, kind="ExternalInput").ap()
    cwk_d = nc.dram_tensor("cwk", [T, 128], F32, kind="ExternalInput").ap()
    swk_d = nc.dram_tensor("swk", [T, 128], F32, kind="ExternalInput").ap()
    tri_d = nc.dram_tensor("tri", [128, 128], F32R, kind="ExternalInput").ap()
    onec_d = nc.dram_tensor("onec", [128, 1], F32R, kind="ExternalInput").ap()
    oner_d = nc.dram_tensor("oner", [1, 128], F32R, kind="ExternalInput").ap()
    id_d = nc.dram_tensor("ident", [128, 128], F32, kind="ExternalInput").ap()
    out_d = nc.dram_tensor("out", [T, HIDDEN], F32, kind="ExternalOutput").ap()

    with tile.TileContext(nc) as tc, ExitStack() as ctx:
        consts = ctx.enter_context(tc.tile_pool(name="consts", bufs=1))
        tri = consts.tile([128, 128], F32R, tag="tri", name="tri")
        nc.sync.dma_start(out=tri[:], in_=tri_d[:])
        ident = consts.tile([128, 128], F32, tag="ident", name="ident")
        nc.sync.dma_start(out=ident[:], in_=id_d[:])
        onec = consts.tile([128, 1], F32R, tag="onec", name="onec")
        nc.sync.dma_start(out=onec[:], in_=onec_d[:])
        oner = consts.tile([1, 128], F32R, tag="oner", name="oner")
        nc.sync.dma_start(out=oner[:], in_=oner_d[:])
        zero_b = consts.tile([128, 1], F32, tag="zerob", name="zerob")
        nc.vector.memset(zero_b[:], 0.0)
        eps_b = consts.tile([128, 1], F32, tag="epsb", name="epsb")
        nc.vector.memset(eps_b[:], EPS)

        # persistent attention operands
        kT_pool = ctx.enter_context(tc.tile_pool(name="kTp", bufs=1))
        kT = kT_pool.tile([128, T], F32R, tag="kT", name="kT")
        v_pool = ctx.enter_context(tc.tile_pool(name="vp", bufs=NT))
        v_tiles = [v_pool.tile([128, 128], F32R, tag="v", name="v")
                   for _ in range(NT)]
        qTc_pool = ctx.enter_context(tc.tile_pool(name="qTcp", bufs=2 * QH))
        oTc_pool = ctx.enter_context(tc.tile_pool(name="oTcp", bufs=6))
        qTc = {}          # (h, c) -> sbuf tile [128d, 512tq]

        # DRAM spill for o^T chunks (keeps Wo out of SBUF until o_proj)
        dram = ctx.enter_context(tc.tile_pool(name="dram", bufs=QH * NCH,
                                              space="DRAM"))
        oTd = {(h, c): dram.tile([128, 512], F32R, tag="oTd", name="oTd")
               for c in range(NCH) for h in range(QH)}

        h_pool = ctx.enter_context(tc.tile_pool(name="hst", bufs=4))
        tabs = ctx.enter_context(tc.tile_pool(name="tabs", bufs=2))
        work = ctx.enter_context(tc.tile_pool(name="work", bufs=2))
        pexp_pool = ctx.enter_context(tc.tile_pool(name="pexpp", bufs=4))
        oev_pool = ctx.enter_context(tc.tile_pool(name="oevp", bufs=2))
        qraw_pool = ctx.enter_context(tc.tile_pool(name="qrawp", bufs=3))

        # 4 PSUM pools x 2 bufs = 8 banks.  Slot rings are shared across
        # phases via one tag per pool; every alloc's release depends only on
        # earlier-emitted instructions (attention is interleaved per chunk),
        # so the rings stay acyclic.
        ps_big = ctx.enter_context(tc.tile_pool(name="ps_big", bufs=2, space="PSUM"))
        ps_acc = ctx.enter_context(tc.tile_pool(name="ps_acc", bufs=2, space="PSUM"))
        ps_aux = ctx.enter_context(tc.tile_pool(name="ps_aux", bufs=2, space="PSUM"))
        ps_opj = ctx.enter_context(tc.tile_pool(name="ps_opj", bufs=2, space="PSUM"))

        def emit_attention_chunk(c):
            ntk = 4 * c + 4
            for h in range(QH):
                po = ps_acc.tile([128, 512], F32, tag="acc", name="po")
                pden = ps_aux.tile([1, 512], F32, tag="aux", name="pden")
                for i in range(ntk):
                    # columns tq < tk-start are causally dead: compute only
                    # [off:512] and never feed the dead prefix to den/PV
                    off = (i - 4 * c) * 128 if i >= 4 * c else 0
                    pss = ps_big.tile([128, 512], F32, tag="big", name="ps")
                    nc.tensor.matmul(pss[:, off:512],
                                     kT[:, i * 128:(i + 1) * 128],
                                     qTc[(h, c)][:, off:512],
                                     start=True, stop=True)
                    pe = pexp_pool.tile([128, 512], F32R, tag="pexp", name="pexp")
                    nc.scalar.activation(pe[:, off:512], pss[:, off:512],
                                         AF.Exp, bias=zero_b[:])
                    if i >= 4 * c:
                        nc.vector.tensor_mul(pe[:, off:off + 128],
                                             pe[:, off:off + 128], tri[:])
                    nc.tensor.matmul(pden[:, off:512], onec[:],
                                     pe[:, off:512],
                                     start=(i == 0), stop=(i == ntk - 1))
                    nc.tensor.matmul(po[:, off:512], v_tiles[i][:],
                                     pe[:, off:512],
                                     start=(i == 0), stop=(i == ntk - 1))
                rden = work.tile([1, 512], F32R, tag="rden", name="rden")
                with nc.allow_low_precision(reason="f32r storage; rounded at matmul"):
                    nc.vector.reciprocal(rden[:], pden[:])
                pbc = ps_aux.tile([128, 512], F32, tag="aux", name="pbc")
                nc.tensor.matmul(pbc[:], oner[:],
                                 rden[:], start=True, stop=True)
                rb = work.tile([128, 512], F32, tag="rb", name="rb")
                nc.scalar.copy(rb[:], pbc[:])
                ot = oTc_pool.tile([128, 512], F32R, tag="oTc", name="oTc")
                nc.vector.tensor_mul(ot[:], po[:], rb[:])
                nc.sync.dma_start(out=oTd[(h, c)][:], in_=ot[:])

        # ---------------- phase 1: QKV + norm + rope + transposes, with
        # attention emitted per completed 512-chunk.  Wqkv resident only here.
        ph12 = ExitStack()
        wq_pool = ph12.enter_context(tc.tile_pool(name="wqp", bufs=NH))
        wq_tiles = []
        for h in range(NH):
            wqt = wq_pool.tile([128, 768], F32R, tag="wqkv", name="wqkv")
            nc.sync.dma_start(out=wqt[:], in_=wq_d[h * 128:(h + 1) * 128, :])
            wq_tiles.append(wqt)

        for tg in range(NT // 2):
            pas = [ps_big.tile([128, 512], F32, tag="big", name="pqa")
                   for _ in range(2)]
            pbs = [ps_acc.tile([128, 256], F32, tag="acc", name="pqb")
                   for _ in range(2)]
            for h in range(NH):
                ht = h_pool.tile([128, 256], F32R, tag="ht", name="ht")
                nc.sync.dma_start(out=ht[:],
                                  in_=hT_d[h * 128:(h + 1) * 128,
                                           tg * 256:(tg + 1) * 256])
                for tt in range(2):
                    lhsT = ht[:, tt * 128:(tt + 1) * 128]
                    nc.tensor.matmul(pas[tt][:], lhsT,
                                     wq_tiles[h][:, 0:512],
                                     start=(h == 0), stop=(h == NH - 1))
                    nc.tensor.matmul(pbs[tt][:], lhsT,
                                     wq_tiles[h][:, 512:768],
                                     start=(h == 0), stop=(h == NH - 1))
            for tt in range(2):
                t = tg * 2 + tt
                c = t // 4
                qr = qraw_pool.tile([128, 512], F32, tag="qraw", name="qraw")
                nc.scalar.copy(qr[:], pas[tt][:])
                kr = qraw_pool.tile([128, 128], F32, tag="kraw", name="kraw")
                nc.vector.tensor_copy(kr[:], pbs[tt][:, 0:128])
                nc.vector.tensor_copy(v_tiles[t][:], pbs[tt][:, 128:256])

                cwq_t = tabs.tile([128, 128], F32, tag="cwq", name="cwq")
                nc.sync.dma_start(out=cwq_t[:], in_=cwq_d[t * 128:(t + 1) * 128, :])
                swq_t = tabs.tile([128, 128], F32, tag="swq", name="swq")
                nc.sync.dma_start(out=swq_t[:], in_=swq_d[t * 128:(t + 1) * 128, :])
                cwk_t = tabs.tile([128, 128], F32, tag="cwk", name="cwk")
                nc.sync.dma_start(out=cwk_t[:], in_=cwk_d[t * 128:(t + 1) * 128, :])
                swk_t = tabs.tile([128, 128], F32, tag="swk", name="swk")
                nc.sync.dma_start(out=swk_t[:], in_=swk_d[t * 128:(t + 1) * 128, :])

                # rms norm scales: ss[:, j] = sum_d x^2 (4 q heads + k)
                ss = work.tile([128, 8], F32, tag="ss", name="ss")
                sqs = work.tile([128, 512], F32, tag="sqs", name="sqs")
                for h in range(QH):
                    nc.scalar.activation(sqs[:, h * 128:(h + 1) * 128],
                                         qr[:, h * 128:(h + 1) * 128],
                                         AF.Square, bias=zero_b[:],
                                         accum_out=ss[:, h:h + 1])
                ksq = work.tile([128, 128], F32, tag="ksq", name="ksq")
                nc.scalar.activation(ksq[:], kr[:], AF.Square, bias=zero_b[:],
                                     accum_out=ss[:, 4:5])
                rt = work.tile([128, 8], F32, tag="rt", name="rt")
                nc.scalar.activation(rt[:, 0:5], ss[:, 0:5], AF.Sqrt,
                                     bias=eps_b[:], scale=1.0 / HEAD_DIM)
                rr = work.tile([128, 8], F32, tag="rr", name="rr")
                nc.vector.reciprocal(rr[:, 0:5], rt[:, 0:5])

                qs = work.tile([128, 512], F32, tag="qs", name="qs")
                for h in range(QH):
                    nc.vector.tensor_scalar_mul(qs[:, h * 128:(h + 1) * 128],
                                                qr[:, h * 128:(h + 1) * 128],
                                                rr[:, h:h + 1])
                ks = work.tile([128, 128], F32, tag="ks", name="ks")
                nc.vector.tensor_scalar_mul(ks[:], kr[:], rr[:, 4:5])

                # rope: y = qs*cw + swap_halves(qs)*sw  (tables broadcast x4)
                qy = work.tile([128, 512], F32, tag="qy", name="qy")
                m2 = work.tile([128, 512], F32, tag="m2", name="m2")
                qs4 = qs[:].rearrange("p (h e) -> p h e", h=QH)
                qy4 = qy[:].rearrange("p (h e) -> p h e", h=QH)
                cwb = cwq_t[:].unsqueeze(1).to_broadcast([128, QH, 128])
                nc.vector.tensor_mul(qy4, qs4, cwb)
                qsv = qs[:].rearrange("p (h j e) -> p h j e", h=QH, j=2)
                m2v = m2[:].rearrange("p (h j e) -> p h j e", h=QH, j=2)
                swv = swq_t[:].rearrange("p (j e) -> p j e", j=2)
                nc.vector.tensor_mul(
                    m2v[:, :, 0, :], qsv[:, :, 1, :],
                    swv[:, 0, :].unsqueeze(1).to_broadcast([128, QH, HALF]))
                nc.vector.tensor_mul(
                    m2v[:, :, 1, :], qsv[:, :, 0, :],
                    swv[:, 1, :].unsqueeze(1).to_broadcast([128, QH, HALF]))
                nc.vector.tensor_add(qy[:], qy[:], m2[:])

                ky = work.tile([128, 128], F32, tag="ky", name="ky")
                km2 = work.tile([128, 128], F32, tag="km2", name="km2")
                nc.vector.tensor_mul(ky[:], ks[:], cwk_t[:])
                ksv = ks[:].rearrange("p (j e) -> p j e", j=2)
                km2v = km2[:].rearrange("p (j e) -> p j e", j=2)
                swkv = swk_t[:].rearrange("p (j e) -> p j e", j=2)
                nc.vector.tensor_mul(km2v[:, 0, :], ksv[:, 1, :], swkv[:, 0, :])
                nc.vector.tensor_mul(km2v[:, 1, :], ksv[:, 0, :], swkv[:, 1, :])
                nc.vector.tensor_add(ky[:], ky[:], km2[:])

                # transpose q heads / k into [d, t] layout via PE
                for h in range(QH):
                    if (h, c) not in qTc:
                        qTc[(h, c)] = qTc_pool.tile([128, 512], F32R,
                                                    tag="qTc", name="qTc")
                    pt = ps_opj.tile([128, 128], F32, tag="opj", name="pt")
                    nc.tensor.transpose(pt[:], qy[:, h * 128:(h + 1) * 128],
                                        ident[:])
                    nc.scalar.copy(qTc[(h, c)][:, (t - 4 * c) * 128:
                                               (t - 4 * c + 1) * 128], pt[:])
                pt = ps_opj.tile([128, 128], F32, tag="opj", name="pt")
                nc.tensor.transpose(pt[:], ky[:], ident[:])
                nc.vector.tensor_copy(kT[:, t * 128:(t + 1) * 128], pt[:])

            if (tg * 2 + 2) % 4 == 0:
                emit_attention_chunk((tg * 2 + 2) // 4 - 1)

        ph12.close()   # release wqkv SBUF before wo allocates

        # ---------------- phase 2: o_proj over spilled o^T
        wo_pool = ctx.enter_context(tc.tile_pool(name="wop", bufs=1))
        wo = wo_pool.tile([128, 4 * HIDDEN], F32R, tag="wo", name="wo")
        for ci in range(4):
            nc.sync.dma_start(out=wo[:, ci * HIDDEN:(ci + 1) * HIDDEN],
                              in_=wo_d[ci * 128:(ci + 1) * 128, :])
        ol_pool = ctx.enter_context(tc.tile_pool(name="olp", bufs=2 * QH))

        for c in range(NCH):
            oL = []
            for h in range(QH):
                o_l = ol_pool.tile([128, 512], F32R, tag="oL", name="oL")
                nc.sync.dma_start(out=o_l[:], in_=oTd[(h, c)][:])
                oL.append(o_l)
            for tt in range(4):
                t = 4 * c + tt
                for n in range(HIDDEN // 512):
                    pso = ps_opj.tile([128, 512], F32, tag="opj", name="psq")
                    for ci in range(QH):
                        nc.tensor.matmul(
                            pso[:],
                            oL[ci][:, tt * 128:(tt + 1) * 128],
                            wo[:, ci * HIDDEN + n * 512:
                               ci * HIDDEN + (n + 1) * 512],
                            start=(ci == 0), stop=(ci == QH - 1))
                    oe = oev_pool.tile([128, 512], F32, tag="oev", name="oev")
                    if (t * 8 + n) % 2 == 0:
                        nc.scalar.copy(oe[:], pso[:])
                    else:
                        nc.vector.tensor_copy(oe[:], pso[:])
                    nc.sync.dma_start(
                        out=out_d[t * 128:(t + 1) * 128, n * 512:(n + 1) * 512],
                        in_=oe[:])

    nc.compile()
    return nc


def make_tables(positions, w, extra_scale):
    """cw/sw rope tables [T, 128], natural layout, norm weight+scale folded."""
    T = positions.shape[0]
    inv_freq = 1.0 / (ROPE_THETA ** (np.arange(HALF, dtype=np.float64) / HALF))
    ang = positions.astype(np.float64)[:, None] * inv_freq
    cos = np.cos(ang)
    sin = np.sin(ang)
    w = np.asarray(w, np.float64)
    cw = np.empty((T, HEAD_DIM), np.float64)
    sw = np.empty((T, HEAD_DIM), np.float64)
    cw[:, :HALF] = cos * w[None, :HALF]
    cw[:, HALF:] = cos * w[None, HALF:]
    sw[:, :HALF] = -sin * w[None, HALF:]
    sw[:, HALF:] = sin * w[None, :HALF]
    return ((cw * extra_scale).astype(np.float32),
            (sw * extra_scale).astype(np.float32))


_NC_CACHE = {}


def kernel(positions, hidden_states, Wqkv, Wo, q_norm_w, k_norm_w):
    global LAST_RESULTS
    positions = np.asarray(positions)
    hidden_states = np.asarray(hidden_states, np.float32)
    Wqkv = np.asarray(Wqkv, np.float32)
    Wo = np.asarray(Wo, np.float32)
    q_norm_w = np.asarray(q_norm_w, np.float32)
    k_norm_w = np.asarray(k_norm_w, np.float32)

    T = hidden_states.shape[0]
    q_size = N_HEADS * HEAD_DIM
    kv_size = N_KV_HEADS * HEAD_DIM

    if T not in _NC_CACHE:
        _NC_CACHE[T] = build_nc(T)
    nc = _NC_CACHE[T]

    hT = np.ascontiguousarray(hidden_states.T)
    cwq, swq = make_tables(positions, q_norm_w, HEAD_DIM ** -0.5)
    cwk, swk = make_tables(positions, k_norm_w, 1.0)
    tri = np.triu(np.ones((128, 128), np.float32))
    onec = np.ones((128, 1), np.float32)
    oner = np.ones((1, 128), np.float32)
    ident = np.eye(128, dtype=np.float32)

    in_maps = []
    for g in range(N_CORES):
        wqkv_g = np.ascontiguousarray(np.concatenate([
            Wqkv[:, 512 * g:512 * (g + 1)],
            Wqkv[:, q_size + 128 * g:q_size + 128 * (g + 1)],
            Wqkv[:, q_size + kv_size + 128 * g:q_size + kv_size + 128 * (g + 1)],
        ], axis=1))
        wo_g = np.ascontiguousarray(Wo[512 * g:512 * (g + 1), :])
        in_maps.append({
            "hT": hT, "wqkv": wqkv_g, "wo": wo_g,
            "cwq": cwq, "swq": swq, "cwk": cwk, "swk": swk,
            "tri": tri, "onec": onec, "oner": oner, "ident": ident,
        })

    res = run_bass_kernel_spmd(nc, in_maps, list(range(N_CORES)), trace=TRACE)
    LAST_RESULTS = res
    acc = np.zeros((T, HIDDEN), np.float64)
    for r in res.results:
        acc += r["out"].astype(np.float64)
    return acc.astype(np.float32)


# revision 11
# speedup vs baseline: 1.0342x; 1.0342x over previous
"""Trainium2 Bass kernel for Mixtral-style attention (QKV proj + per-head
RMSNorm on Q/K + NeoX RoPE + GQA causal attention + output projection).

Sharding: tensor-parallel across heads — each of the 8 cores owns one GQA
group (4 Q heads + 1 KV head): Wqkv column-sharded, Wo row-sharded.  The
o_proj partial sums are reduced on the host (the natural "unshard" for
row-parallel Wo).

Device-side layout strategy per core:
  - host supplies hidden^T [H, T] so the QKV matmul contracts on partitions
    with zero on-device transposes of the activations
  - qkv computed in natural [t, c] layout -> RMSNorm reduction + RoPE pair
    shuffle live on the free axis (cheap DVE ops)
  - q/k transposed per 128-tile via the PE (identity matmul) into [d, t]
    layout; attention computes S^T = k_eff^T-contract-q_eff directly, so
    softmax normalization needs no P transposes:
      P = exp(S^T) (causal-masked), den = ones^T @ P (PE),
      oT = v^T-contract-P (PE), o = oT * (1/den broadcast via K=1 matmul)
  - no max-subtraction in softmax: post-RMSNorm scores are bounded by
    sqrt(d)*max|w|^2*scale ~= 12, far from fp32 overflow, and softmax is
    shift-invariant so the result is identical
  - matmuls run as float32r (TF32-like, 1 col/cycle at N>=256; 4x faster
    than fp32) with fp32 PSUM accumulation
"""

import numpy as np
from contextlib import ExitStack

import concourse.bass as bass
import concourse.tile as tile
from concourse import bacc, mybir
from concourse.bass_utils import run_bass_kernel_spmd

F32 = mybir.dt.float32
F32R = mybir.dt.float32r
AF = mybir.ActivationFunctionType

HIDDEN = 4096
HEAD_DIM = 128
N_HEADS = 32
N_KV_HEADS = 8
N_CORES = 8
QH = N_HEADS // N_CORES          # 4 q heads per core
ROPE_THETA = 1e6
EPS = 1e-6
HALF = HEAD_DIM // 2
NH = HIDDEN // 128               # 32 contraction tiles

# set by test harness to collect profile info
TRACE = False
LAST_RESULTS = None


def build_nc(T):
    NT = T // 128                 # t tiles
    NCH = T // 512                # 512-wide tq chunks
    assert T % 512 == 0

    nc = bacc.Bacc("TRN2", target_bir_lowering=False, debug=False,
                   num_devices=N_CORES)

    hT_d = nc.dram_tensor("hT", [HIDDEN, T], F32R, kind="ExternalInput").ap()
    wq_d = nc.dram_tensor("wqkv", [HIDDEN, 768], F32R, kind="ExternalInput").ap()
    wo_d = nc.dram_tensor("wo", [512, HIDDEN], F32R, kind="ExternalInput").ap()
    cwq_d = nc.dram_tensor("cwq", [T, 128], F32, kind="ExternalInput").ap()
    swq_d = nc.dram_tensor("swq", [T, 128], F32, kind="ExternalInput").ap()
    cwk_d = nc.dram_tensor("cwk", [T, 128], F32, kind="ExternalInput").ap()
    swk_d = nc.dram_tensor("swk", [T, 128], F32, kind="ExternalInput").ap()
    tri_d = nc.dram_tensor("tri", [128, 128], F32R, kind="ExternalInput").ap()
    onec_d = nc.dram_tensor("onec", [128, 1], F32R, kind="ExternalInput").ap()
    oner_d = nc.dram_tensor("oner", [1, 128], F32R, kind="ExternalInput").ap()
    id_d = nc.dram_tensor("ident", [128, 128], F32, kind="ExternalInput").ap()
    out_d = nc.dram_tensor("out", [T, HIDDEN], F32, kind="ExternalOutput").ap()

    with tile.TileContext(nc) as tc, ExitStack() as ctx:
        consts = ctx.enter_context(tc.tile_pool(name="consts", bufs=1))
        tri = consts.tile([128, 128], F32R, tag="tri", name="tri")
        nc.sync.dma_start(out=tri[:], in_=tri_d[:])
        ident = consts.tile([128, 128], F32, tag="ident", name="ident")
        nc.sync.dma_start(out=ident[:], in_=id_d[:])
        onec = consts.tile([128, 1], F32R, tag="onec", name="onec")
        nc.sync.dma_start(out=onec[:], in_=onec_d[:])
        oner = consts.tile([1, 128], F32R, tag="oner", name="oner")
        nc.sync.dma_start(out=oner[:], in_=oner_d[:])
        zero_b = consts.tile([128, 1], F32, tag="zerob", name="zerob")
        nc.vector.memset(zero_b[:], 0.0)
        eps_b = consts.tile([128, 1], F32, tag="epsb", name="epsb")
        nc.vector.memset(eps_b[:], EPS)

        # persistent attention operands
        kT_pool = ctx.enter_context(tc.tile_pool(name="kTp", bufs=1))
        kT = kT_pool.tile([128, T], F32R, tag="kT", name="kT")
        v_pool = ctx.enter_context(tc.tile_pool(name="vp", bufs=NT))
        v_tiles = [v_pool.tile([128, 128], F32R, tag="v", name="v")
                   for _ in range(NT)]
        qTc_pool = ctx.enter_context(tc.tile_pool(name="qTcp", bufs=13))
        oTc_pool = ctx.enter_context(tc.tile_pool(name="oTcp", bufs=5))
        qTc = {}          # (h, c) -> sbuf tile [128d, 512tq]

        # DRAM spill for o^T chunks (keeps Wo out of SBUF until o_proj)
        dram = ctx.enter_context(tc.tile_pool(name="dram", bufs=QH * NCH,
                                              space="DRAM"))
        oTd = {(h, c): dram.tile([128, 512], F32R, tag="oTd", name="oTd")
               for c in range(NCH) for h in range(QH)}

        h_pool = ctx.enter_context(tc.tile_pool(name="hst", bufs=4))
        tabs = ctx.enter_context(tc.tile_pool(name="tabs", bufs=2))
        work = ctx.enter_context(tc.tile_pool(name="work", bufs=2))
        pexp_pool = ctx.enter_context(tc.tile_pool(name="pexpp", bufs=3))
        oev_pool = ctx.enter_context(tc.tile_pool(name="oevp", bufs=2))
        qraw_pool = ctx.enter_context(tc.tile_pool(name="qrawp", bufs=2))

        # 4 PSUM pools x 2 bufs = 8 banks.  Slot rings are shared across
        # phases via one tag per pool; every alloc's release depends only on
        # earlier-emitted instructions (attention is interleaved per chunk),
        # so the rings stay acyclic.
        ps_big = ctx.enter_context(tc.tile_pool(name="ps_big", bufs=2, space="PSUM"))
        ps_acc = ctx.enter_context(tc.tile_pool(name="ps_acc", bufs=2, space="PSUM"))
        ps_aux = ctx.enter_context(tc.tile_pool(name="ps_aux", bufs=2, space="PSUM"))
        ps_opj = ctx.enter_context(tc.tile_pool(name="ps_opj", bufs=2, space="PSUM"))

        def emit_attention_chunk(c):
            ntk = 4 * c + 4
            for h in range(QH):
                po = ps_acc.tile([128, 512], F32, tag="acc", name="po")
                pden = ps_aux.tile([1, 512], F32, tag="aux", name="pden")
                for i in range(ntk):
                    # columns tq < tk-start are causally dead: compute only
                    # [off:512] and never feed the dead prefix to den/PV
                    off = (i - 4 * c) * 128 if i >= 4 * c else 0
                    pss = ps_big.tile([128, 512], F32, tag="big", name="ps")
                    nc.tensor.matmul(pss[:, off:512],
                                     kT[:, i * 128:(i + 1) * 128],
                                     qTc[(h, c)][:, off:512],
                                     start=True, stop=True)
                    pe = pexp_pool.tile([128, 512], F32R, tag="pexp", name="pexp")
                    nc.scalar.activation(pe[:, off:512], pss[:, off:512],
                                         AF.Exp, bias=zero_b[:])
                    if i >= 4 * c:
                        nc.vector.tensor_mul(pe[:, off:off + 128],
                                             pe[:, off:off + 128], tri[:])
                    nc.tensor.matmul(pden[:, off:512], onec[:],
                                     pe[:, off:512],
                                     start=(i == 0), stop=(i == ntk - 1))
                    nc.tensor.matmul(po[:, off:512], v_tiles[i][:],
                                     pe[:, off:512],
                                     start=(i == 0), stop=(i == ntk - 1))
                rden = work.tile([1, 512], F32R, tag="rden", name="rden")
                with nc.allow_low_precision(reason="f32r storage; rounded at matmul"):
                    nc.vector.reciprocal(rden[:], pden[:])
                pbc = ps_opj.tile([128, 512], F32, tag="opj", name="pbc")
                nc.tensor.matmul(pbc[:], oner[:],
                                 rden[:], start=True, stop=True)
                rb = work.tile([128, 512], F32, tag="rb", name="rb")
                nc.scalar.copy(rb[:], pbc[:])
                ot = oTc_pool.tile([128, 512], F32R, tag="oTc", name="oTc")
                nc.vector.tensor_mul(ot[:], po[:], rb[:])
                nc.sync.dma_start(out=oTd[(h, c)][:], in_=ot[:])

        # ---------------- phase 1: QKV + norm + rope + transposes, with
        # attention emitted per completed 512-chunk.  Wqkv resident only here.
        ph12 = ExitStack()
        wq_pool = ph12.enter_context(tc.tile_pool(name="wqp", bufs=1))
        wq = wq_pool.tile([128, NH * 768], F32R, tag="wqkv", name="wqkv")
        for h in range(NH):
            nc.sync.dma_start(out=wq[:, h * 768:(h + 1) * 768],
                              in_=wq_d[h * 128:(h + 1) * 128, :])

        for tg in range(NT // 2):
            pas = [ps_big.tile([128, 512], F32, tag="big", name="pqa")
                   for _ in range(2)]
            pbs = [ps_acc.tile([128, 256], F32, tag="acc", name="pqb")
                   for _ in range(2)]
            for h in range(NH):
                ht = h_pool.tile([128, 256], F32R, tag="ht", name="ht")
                nc.sync.dma_start(out=ht[:],
                                  in_=hT_d[h * 128:(h + 1) * 128,
                                           tg * 256:(tg + 1) * 256])
                for tt in range(2):
                    lhsT = ht[:, tt * 128:(tt + 1) * 128]
                    nc.tensor.matmul(pas[tt][:], lhsT,
                                     wq[:, h * 768:h * 768 + 512],
                                     start=(h == 0), stop=(h == NH - 1))
                    nc.tensor.matmul(pbs[tt][:], lhsT,
                                     wq[:, h * 768 + 512:(h + 1) * 768],
                                     start=(h == 0), stop=(h == NH - 1))
            for tt in range(2):
                t = tg * 2 + tt
                c = t // 4
                qr = qraw_pool.tile([128, 512], F32, tag="qraw", name="qraw")
                nc.scalar.copy(qr[:], pas[tt][:])
                kr = qraw_pool.tile([128, 128], F32, tag="kraw", name="kraw")
                nc.vector.tensor_copy(kr[:], pbs[tt][:, 0:128])
                nc.vector.tensor_copy(v_tiles[t][:], pbs[tt][:, 128:256])

                cwq_t = tabs.tile([128, 128], F32, tag="cwq", name="cwq")
                nc.sync.dma_start(out=cwq_t[:], in_=cwq_d[t * 128:(t + 1) * 128, :])
                swq_t = tabs.tile([128, 128], F32, tag="swq", name="swq")
                nc.sync.dma_start(out=swq_t[:], in_=swq_d[t * 128:(t + 1) * 128, :])
                cwk_t = tabs.tile([128, 128], F32, tag="cwk", name="cwk")
                nc.sync.dma_start(out=cwk_t[:], in_=cwk_d[t * 128:(t + 1) * 128, :])
                swk_t = tabs.tile([128, 128], F32, tag="swk", name="swk")
                nc.sync.dma_start(out=swk_t[:], in_=swk_d[t * 128:(t + 1) * 128, :])

                # rms norm scales: ss[:, j] = sum_d x^2 (4 q heads + k)
                ss = work.tile([128, 8], F32, tag="ss", name="ss")
                sqs = work.tile([128, 512], F32, tag="sqs", name="sqs")
                for h in range(QH):
                    nc.scalar.activation(sqs[:, h * 128:(h + 1) * 128],
                                         qr[:, h * 128:(h + 1) * 128],
                                         AF.Square, bias=zero_b[:],
                                         accum_out=ss[:, h:h + 1])
                ksq = work.tile([128, 128], F32, tag="ksq", name="ksq")
                nc.scalar.activation(ksq[:], kr[:], AF.Square, bias=zero_b[:],
                                     accum_out=ss[:, 4:5])
                rt = work.tile([128, 8], F32, tag="rt", name="rt")
                nc.scalar.activation(rt[:, 0:5], ss[:, 0:5], AF.Sqrt,
                                     bias=eps_b[:], scale=1.0 / HEAD_DIM)
                rr = work.tile([128, 8], F32, tag="rr", name="rr")
                nc.vector.reciprocal(rr[:, 0:5], rt[:, 0:5])

                qs = work.tile([128, 512], F32, tag="qs", name="qs")
                for h in range(QH):
                    nc.vector.tensor_scalar_mul(qs[:, h * 128:(h + 1) * 128],
                                                qr[:, h * 128:(h + 1) * 128],
                                                rr[:, h:h + 1])
                ks = work.tile([128, 128], F32, tag="ks", name="ks")
                nc.vector.tensor_scalar_mul(ks[:], kr[:], rr[:, 4:5])

                # rope: y = qs*cw + swap_halves(qs)*sw  (tables broadcast x4)
                qy = work.tile([128, 512], F32, tag="qy", name="qy")
                m2 = work.tile([128, 512], F32, tag="m2", name="m2")
                qs4 = qs[:].rearrange("p (h e) -> p h e", h=QH)
                qy4 = qy[:].rearrange("p (h e) -> p h e", h=QH)
                cwb = cwq_t[:].unsqueeze(1).to_broadcast([128, QH, 128])
                nc.vector.tensor_mul(qy4, qs4, cwb)
                qsv = qs[:].rearrange("p (h j e) -> p h j e", h=QH, j=2)
                m2v = m2[:].rearrange("p (h j e) -> p h j e", h=QH, j=2)
                swv = swq_t[:].rearrange("p (j e) -> p j e", j=2)
                nc.vector.tensor_mul(
                    m2v[:, :, 0, :], qsv[:, :, 1, :],
                    swv[:, 0, :].unsqueeze(1).to_broadcast([128, QH, HALF]))
                nc.vector.tensor_mul(
                    m2v[:, :, 1, :], qsv[:, :, 0, :],
                    swv[:, 1, :].unsqueeze(1).to_broadcast([128, QH, HALF]))
                nc.vector.tensor_add(qy[:], qy[:], m2[:])

                ky = work.tile([128, 128], F32, tag="ky", name="ky")
                km2 = work.tile([128, 128], F32, tag="km2", name="km2")
                nc.vector.tensor_mul(ky[:], ks[:], cwk_t[:])
                ksv = ks[:].rearrange("p (j e) -> p j e", j=2)
                km2v = km2[:].rearrange("p (j e) -> p j e", j=2)
                swkv = swk_t[:].rearrange("p (j e) -> p j e", j=2)
                nc.vector.tensor_mul(km2v[:, 0, :], ksv[:, 1, :], swkv[:, 0, :])
                nc.vector.tensor_mul(km2v[:, 1, :], ksv[:, 0, :], swkv[:, 1, :])
                nc.vector.tensor_add(ky[:], ky[:], km2[:])

                # transpose q heads / k into [d, t] layout via PE
                for h in range(QH):
                    if (h, c) not in qTc:
                        qTc[(h, c)] = qTc_pool.tile([128, 512], F32R,
                                                    tag="qTc", name="qTc")
                    pt = ps_opj.tile([128, 128], F32, tag="opj", name="pt")
                    nc.tensor.transpose(pt[:], qy[:, h * 128:(h + 1) * 128],
                                        ident[:])
                    nc.scalar.copy(qTc[(h, c)][:, (t - 4 * c) * 128:
                                               (t - 4 * c + 1) * 128], pt[:])
                pt = ps_opj.tile([128, 128], F32, tag="opj", name="pt")
                nc.tensor.transpose(pt[:], ky[:], ident[:])
                nc.vector.tensor_copy(kT[:, t * 128:(t + 1) * 128], pt[:])

            if (tg * 2 + 2) % 4 == 0:
                done = (tg * 2 + 2) // 4    # chunks fully produced
                if done >= 2:
                    emit_attention_chunk(done - 2)
        emit_attention_chunk(NCH - 1)

        ph12.close()   # release wqkv SBUF before wo allocates

        # ---------------- phase 2: o_proj over spilled o^T
        wo_pool = ctx.enter_context(tc.tile_pool(name="wop", bufs=1))
        wo = wo_pool.tile([128, 4 * HIDDEN], F32R, tag="wo", name="wo")
        for ci in range(4):
            nc.sync.dma_start(out=wo[:, ci * HIDDEN:(ci + 1) * HIDDEN],
                              in_=wo_d[ci * 128:(ci + 1) * 128, :])
        ol_pool = ctx.enter_context(tc.tile_pool(name="olp", bufs=2 * QH))

        for c in range(NCH):
            oL = []
            for h in range(QH):
                o_l = ol_pool.tile([128, 512], F32R, tag="oL", name="oL")
                nc.sync.dma_start(out=o_l[:], in_=oTd[(h, c)][:])
                oL.append(o_l)
            for tt in range(4):
                t = 4 * c + tt
                for n in range(HIDDEN // 512):
                    pso = ps_opj.tile([128, 512], F32, tag="opj", name="psq")
                    for ci in range(QH):
                        nc.tensor.matmul(
                            pso[:],
                            oL[ci][:, tt * 128:(tt + 1) * 128],
                            wo[:, ci * HIDDEN + n * 512:
                               ci * HIDDEN + (n + 1) * 512],
                            start=(ci == 0), stop=(ci == QH - 1))
                    oe = oev_pool.tile([128, 512], F32, tag="oev", name="oev")
                    if (t * 8 + n) % 2 == 0:
                        nc.scalar.copy(oe[:], pso[:])
                    else:
                        nc.vector.tensor_copy(oe[:], pso[:])
                    nc.sync.dma_start(
                        out=out_d[t * 128:(t + 1) * 128, n * 512:(n + 1) * 512],
                        in_=oe[:])

    nc.compile()
    return nc


def make_tables(positions, w, extra_scale):
    """cw/sw rope tables [T, 128], natural layout, norm weight+scale folded."""
    T = positions.shape[0]
    inv_freq = 1.0 / (ROPE_THETA ** (np.arange(HALF, dtype=np.float64) / HALF))
    ang = positions.astype(np.float64)[:, None] * inv_freq
    cos = np.cos(ang)
    sin = np.sin(ang)
    w = np.asarray(w, np.float64)
    cw = np.empty((T, HEAD_DIM), np.float64)
    sw = np.empty((T, HEAD_DIM), np.float64)
    cw[:, :HALF] = cos * w[None, :HALF]
    cw[:, HALF:] = cos * w[None, HALF:]
    sw[:, :HALF] = -sin * w[None, HALF:]
    sw[:, HALF:] = sin * w[None, :HALF]
    return ((cw * extra_scale).astype(np.float32),
            (sw * extra_scale).astype(np.float32))


_NC_CACHE = {}


def kernel(positions, hidden_states, Wqkv, Wo, q_norm_w, k_norm_w):
    global LAST_RESULTS
    positions = np.asarray(positions)
    hidden_states = np.asarray(hidden_states, np.float32)
    Wqkv = np.asarray(Wqkv, np.float32)
    Wo = np.asarray(Wo, np.float32)
    q_norm_w = np.asarray(q_norm_w, np.float32)
    k_norm_w = np.asarray(k_norm_w, np.float32)

    T = hidden_states.shape[0]
    q_size = N_HEADS * HEAD_DIM
    kv_size = N_KV_HEADS * HEAD_DIM

    if T not in _NC_CACHE:
        _NC_CACHE[T] = build_nc(T)
    nc = _NC_CACHE[T]

    hT = np.ascontiguousarray(hidden_states.T)
    cwq, swq = make_tables(positions, q_norm_w, HEAD_DIM ** -0.5)
    cwk, swk = make_tables(positions, k_norm_w, 1.0)
    tri = np.triu(np.ones((128, 128), np.float32))
    onec = np.ones((128, 1), np.float32)
    oner = np.ones((1, 128), np.float32)
    ident = np.eye(128, dtype=np.float32)

    in_maps = []
    for g in range(N_CORES):
        wqkv_g = np.ascontiguousarray(np.concatenate([
            Wqkv[:, 512 * g:512 * (g + 1)],
            Wqkv[:, q_size + 128 * g:q_size + 128 * (g + 1)],
            Wqkv[:, q_size + kv_size + 128 * g:q_size + kv_size + 128 * (g + 1)],
        ], axis=1))
        wo_g = np.ascontiguousarray(Wo[512 * g:512 * (g + 1), :])
        in_maps.append({
            "hT": hT, "wqkv": wqkv_g, "wo": wo_g,
            "cwq": cwq, "swq": swq, "cwk": cwk, "swk": swk,
            "tri": tri, "onec": onec, "oner": oner, "ident": ident,
        })

    res = run_bass_kernel_spmd(nc, in_maps, list(range(N_CORES)), trace=TRACE)
    LAST_RESULTS = res
    acc = np.zeros((T, HIDDEN), np.float64)
    for r in res.results:
        acc += r["out"].astype(np.float64)
    return acc.astype(np.float32)
